# revision 6
# baseline (speedup 1.0000x reference)
"""2-layer LSTM (B=128, T=256, D=512, H=1024) + linear head + ELU on 8 trn2 cores.

Strategy (all hardcoded; v2 "feature-major" design):
  - Everything is computed TRANSPOSED (feature-major): gates^T [1024, B],
    h^T [H, B], c^T [H_local, B]. With the full batch B=128 as the matmul
    moving dim and weights as the 128x128 stationary operand, the PE runs at
    full utilization (the v1 batch-major layout wasted half the array), and
    h^T is directly the rhs of the next step's matmul - no PE transposes.
  - Sharding: the two dies are fully redundant (both compute the whole
    batch); within a die, 4-way tensor-parallel over the hidden dim. Each
    core owns a 256-wide hidden slice = 1024 local gate rows (transposed),
    ordered [i_lo f_lo g_lo o_lo | i_hi f_hi g_hi o_hi] x 128 so each PSUM
    bank holds one i/f/g/o quartet. Per step a core broadcasts its h^T slice
    [256, 128] (bf16, 64KB) to the 3 die peers + itself (XOR-relative slots,
    weight rows pre-permuted on host to match). Zero cross-die traffic.
  - No DRAM round-trips: X^T streams in per-step ([128, 512] bf16 tiles,
    prefetched), xg0/xg1 are fused into the per-step matmul accumulation
    (x-chunks first - for layer 0 they don't wait on the h gather - then
    h-chunks). Layer 1 lags layer 0 by one step so its h0 input is already
    gathered when it runs.
  - Gate biases ride the ACT sigmoid/tanh ops as per-partition bias vectors.
  - Gather buffers are mod-3 rings so a peer's step-t+1 write can never race
    my step-t reads. All weights stay SBUF-resident (bf16, ~60KB/partition).
"""

import sys
from contextlib import ExitStack

import ml_dtypes
import numpy as np

for _p in ("/opt/trn_rl_repo", "/root/.axon_site/_ro/trn_rl_repo"):
    if _p not in sys.path:
        sys.path.append(_p)

import concourse.bacc as bacc
import concourse.mybir as mybir
import concourse.tile as tile
from concourse.bass_utils import run_bass_kernel_spmd
from concourse.tile_rust import add_dep_helper

F32 = mybir.dt.float32
BF16 = mybir.dt.bfloat16
AF = mybir.ActivationFunctionType

P = 128
T = 256
D = 512
H = 1024
BR = 256
B = 128    # full batch on every core (dies are redundant)
HL = 256   # hidden units per core
NL = 1024  # local gate rows per core (i|f|g|o quartets, see cols order)
NUM_CORES = 8


def _build(nc, n_steps):
    """Emit the whole program. Returns (inst, sem, val) waits to patch after
    TileContext exit (remote-arrival waits the scheduler can't model)."""
    xt_in = nc.dram_tensor("XT", [n_steps, P, 512], BF16, kind="ExternalInput").ap()
    wx0_in = nc.dram_tensor("Wx0", [D, NL], BF16, kind="ExternalInput").ap()
    wh0_in = nc.dram_tensor("Wh0", [H, NL], BF16, kind="ExternalInput").ap()
    wx1_in = nc.dram_tensor("Wx1", [H, NL], BF16, kind="ExternalInput").ap()
    wh1_in = nc.dram_tensor("Wh1", [H, NL], BF16, kind="ExternalInput").ap()
    wbr_in = nc.dram_tensor("Wbr", [H, BR], BF16, kind="ExternalInput").ap()
    b0_in = nc.dram_tensor("b0p", [P, 8], F32, kind="ExternalInput").ap()
    b1_in = nc.dram_tensor("b1p", [P, 8], F32, kind="ExternalInput").ap()
    bbr_in = nc.dram_tensor("bbrp", [P, 2], F32, kind="ExternalInput").ap()
    y_out = nc.dram_tensor("y", [2, P, B], F32, kind="ExternalOutput").ap()

    # SBUF-resident weights (feature-major: partition = input-row-in-chunk,
    # free = [k-chunk, local gate col])
    sWx0 = nc.alloc_sbuf_tensor("sWx0", [P, 4, NL], BF16).ap()
    sWh0 = nc.alloc_sbuf_tensor("sWh0", [P, 8, NL], BF16).ap()
    sWx1 = nc.alloc_sbuf_tensor("sWx1", [P, 8, NL], BF16).ap()
    sWh1 = nc.alloc_sbuf_tensor("sWh1", [P, 8, NL], BF16).ap()
    sWbr = nc.alloc_sbuf_tensor("sWbr", [P, 8, BR], BF16).ap()
    sB0 = nc.alloc_sbuf_tensor("sB0", [P, 8], F32).ap()
    sB1 = nc.alloc_sbuf_tensor("sB1", [P, 8], F32).ap()
    sBbr = nc.alloc_sbuf_tensor("sBbr", [P, 2], F32).ap()

    # comm buffers (stable addresses for remote writes); mod-3 rings
    gath = [nc.alloc_sbuf_tensor(f"g{l}", [P, 3, 8, B], BF16).ap() for l in range(2)]
    snd = [nc.alloc_sbuf_tensor(f"s{l}", [P, 3, 2, B], BF16).ap() for l in range(2)]
    cst = [nc.alloc_sbuf_tensor(f"c{l}", [P, 2, B], F32).ap() for l in range(2)]

    rsem = [nc.alloc_semaphore("rsem0"), nc.alloc_semaphore("rsem1")]
    lsem = nc.alloc_semaphore("lsem")

    patches = []

    with tile.TileContext(nc) as tc:
        barrier_nop = nc.gpsimd.nop(nofuse=True)

        # weight loads
        for sb, src, nk in ((sWx0, wx0_in, 4), (sWh0, wh0_in, 8),
                            (sWx1, wx1_in, 8), (sWh1, wh1_in, 8)):
            v = src.rearrange("(k p) n -> k p n", p=P)
            for k in range(nk):
                nc.sync.dma_start(out=sb[:, k], in_=v[k])
        wbrv = wbr_in.rearrange("(k p) n -> k p n", p=P)
        for k in range(8):
            nc.sync.dma_start(out=sWbr[:, k], in_=wbrv[k])
        nc.sync.dma_start(out=sB0, in_=b0_in)
        nc.sync.dma_start(out=sB1, in_=b1_in)
        nc.sync.dma_start(out=sBbr, in_=bbr_in)
        nc.vector.memset(cst[0], 0.0)
        nc.vector.memset(cst[1], 0.0)

        stack = ExitStack()
        ps_pool = stack.enter_context(tc.tile_pool(name="psum", bufs=6, space="PSUM"))
        xt_pool = stack.enter_context(tc.tile_pool(name="xtp", bufs=8))
        tmp_pool = stack.enter_context(tc.tile_pool(name="tmp", bufs=8))
        hd_pool = stack.enter_context(tc.tile_pool(name="hdp", bufs=2))
        first_prep = [None]

        def gated_mms(mms_args, wait):
            """Emit a run of matmuls; the first carries `wait` (a (sem, val)
            runtime patch) and the rest are order-chained to it so the
            scheduler can't hoist any of them above the semaphore wait."""
            first = None
            for out, lhsT, rhs, start, stop in mms_args:
                mm = nc.tensor.matmul(out, lhsT, rhs, start=start, stop=stop)
                if first is None:
                    first = mm
                    if wait is not None:
                        patches.append((mm, wait[0], wait[1]))
                else:
                    add_dep_helper(mm.ins, first.ins, sync=False,
                                   reason="mms chained after gated first")
            return first

        def layer_mms(wx, nkx, xin_tiles, x_wait, wh, g_rhs, h_wait, skip_rec):
            """One layer-step's matmuls: 2 psum quartet tiles [P, 512];
            x-chunks first, then h-chunks. Returns [psA, psB]."""
            pss = [ps_pool.tile([P, 512], F32, name="ps") for _ in range(2)]

            def out_ap(j):
                return pss[j // 4][:, 128 * (j % 4) : 128 * (j % 4 + 1)]

            # Exactly ONE start=True per psum bank tile (j==0 / j==4): a
            # start clears has_written for the whole 2KiB bank row, so a
            # second start in the same bank would wipe sibling regions'
            # accumulation state. Later regions' first matmuls rely on the
            # per-element semantics instead (bit clear -> overwrite + set).
            xargs = []
            for j in range(8):
                for kx in range(nkx):
                    xargs.append((out_ap(j), wx[:, kx, 128 * j : 128 * (j + 1)],
                                  xin_tiles[kx], kx == 0 and j % 4 == 0,
                                  kx == nkx - 1 and skip_rec))
            gated_mms(xargs, x_wait)
            if not skip_rec:
                hargs = []
                for j in range(8):
                    for kh in range(8):
                        hargs.append((out_ap(j), wh[:, kh, 128 * j : 128 * (j + 1)],
                                      g_rhs[:, kh], False, kh == 7))
                gated_mms(hargs, h_wait)
            return pss

        def elem_quartet(ps, half, sB, c, sndt):
            """ps: [P, 512] psum quartet [i f g o] for one 128-half of the
            slice; writes h^T (bf16) into sndt[:, half] and updates c."""
            nc.scalar.activation(ps[:, 0:128], ps[:, 0:128], AF.Sigmoid,
                                 bias=sB[:, 4 * half + 0 : 4 * half + 1])
            nc.scalar.activation(ps[:, 128:256], ps[:, 128:256], AF.Sigmoid,
                                 bias=sB[:, 4 * half + 1 : 4 * half + 2])
            gsb = tmp_pool.tile([P, B], F32, name="gsb")
            nc.scalar.activation(gsb, ps[:, 256:384], AF.Tanh,       # -> SBUF:
                                 bias=sB[:, 4 * half + 2 : 4 * half + 3])
            nc.scalar.activation(ps[:, 384:512], ps[:, 384:512], AF.Sigmoid,
                                 bias=sB[:, 4 * half + 3 : 4 * half + 4])
            t1 = tmp_pool.tile([P, B], F32, name="t1")
            nc.vector.tensor_mul(t1, ps[:, 0:128], gsb)              # i * g
            t2 = tmp_pool.tile([P, B], F32, name="t2")
            nc.vector.tensor_mul(t2, ps[:, 128:256], c[:, half])     # f * c
            nc.vector.tensor_add(c[:, half], t1, t2)
            tcn = tmp_pool.tile([P, B], F32, name="tc")
            nc.scalar.activation(tcn, c[:, half], AF.Tanh)
            nc.vector.tensor_mul(sndt[:, half], ps[:, 384:512], tcn)  # o*tanh(c)

        def send_layer(l, slot, sndt):
            for k in range(4):
                rd = [None] * 8
                rd[k] = (0, k)
                prep = nc.gpsimd.remote_dma_broadcast(
                    gath[l][:, slot, 2 * k : 2 * (k + 1)], sndt,
                    rsem[l], lsem, rdests=rd, queue_num=l)
                if first_prep[0] is None:
                    first_prep[0] = prep
                    add_dep_helper(prep.ins, barrier_nop.ins, sync=False,
                                   reason="sends after entry barrier")
            nc.gpsimd.trigger_dma(count=None, queue_num=l)

        def layer_step(l, t, s, wx, nkx, xin_tiles, x_wait, sB):
            """l: layer; t: program step (send slot t%3, gathers read slot
            (t-1)%3); s: layer-local time (h_{s-1} is contracted; s==0 skips
            the recurrent phase)."""
            g_rhs = gath[l][:, (t - 1) % 3]
            h_wait = (rsem[l], 8 * s) if s >= 1 else None
            pss = layer_mms(wx, nkx, xin_tiles, x_wait, sWh0 if l == 0 else sWh1,
                            g_rhs, h_wait, s == 0)
            sndt = snd[l][:, t % 3]
            for half in range(2):
                elem_quartet(pss[half], half, sB, cst[l], sndt)
            send_layer(l, t % 3, sndt)

        # ---------------- main loop ----------------
        for t in range(n_steps):
            # layer 0, time t: x-chunks from the streamed X^T tile
            xt = xt_pool.tile([P, 4, B], BF16, name="xt")
            nc.sync.dma_start(out=xt, in_=xt_in[t].rearrange("p (k b) -> p k b", k=4))
            layer_step(0, t, t, sWx0, 4, [xt[:, kx] for kx in range(4)],
                       None, sB0)
            # layer 1, time t-1: x-input = gathered h0_{t-1}
            if t >= 1:
                g0r = gath[0][:, (t - 1) % 3]
                layer_step(1, t, t - 1, sWx1, 8, [g0r[:, k] for k in range(8)],
                           (rsem[0], 8 * t), sB1)

        # tail: layer 1, time n_steps-1 (program step tn = n_steps)
        tn = n_steps
        g0r = gath[0][:, (tn - 1) % 3]
        layer_step(1, tn, tn - 1, sWx1, 8, [g0r[:, k] for k in range(8)],
                   (rsem[0], 8 * tn), sB1)

        # ---------------- head: y^T = ELU(Wbr @ h1_last + bbr) -------------
        g1r = gath[1][:, tn % 3]
        psh = ps_pool.tile([P, 512], F32, name="ps")
        hargs = []
        for jo in range(2):
            for k in range(8):
                hargs.append((psh[:, 128 * jo : 128 * (jo + 1)],
                              sWbr[:, k, 128 * jo : 128 * (jo + 1)],
                              g1r[:, k], k == 0 and jo == 0, k == 7))
        gated_mms(hargs, (rsem[1], 8 * tn))
        for jo in range(2):
            pc = psh[:, 128 * jo : 128 * (jo + 1)]
            xv = hd_pool.tile([P, B], F32, name="xv")
            nc.scalar.activation(xv, pc, AF.Identity, bias=sBbr[:, jo : jo + 1])
            rl = hd_pool.tile([P, B], F32, name="rl")
            nc.vector.tensor_scalar_max(rl, xv, 0.0)
            mn = hd_pool.tile([P, B], F32, name="mn")
            nc.vector.tensor_scalar_min(mn, xv, 0.0)
            ex = hd_pool.tile([P, B], F32, name="ex")
            nc.scalar.activation(ex, mn, AF.Exp)
            s1 = hd_pool.tile([P, B], F32, name="s1")
            nc.vector.tensor_add(s1, rl, ex)
            yv = hd_pool.tile([P, B], F32, name="yv")
            nc.vector.tensor_scalar_add(yv, s1, -1.0)
            nc.sync.dma_start(out=y_out[jo], in_=yv)
        stack.close()

    # post-Tile patches (scheduler can't model remote increments)
    nc._bir_kernel_barrier_sem_replica_groups.append(set(range(NUM_CORES)))
    barrier_nop.wait_op(nc._bir_kernel_barrier_sem, nc.bir_kernel_barrier_sem_inc,
                        "sem-ge", check=False)
    for inst, sem, val in patches:
        if val > 0:
            inst.wait_op(sem, val, "sem-ge", check=False)
    return patches


def build_program(n_steps=T):
    nc = bacc.Bacc("TRN2", target_bir_lowering=False, debug=False,
                   num_devices=NUM_CORES, num_swdge_queues=2)
    _build(nc, n_steps)
    nc.compile()
    return nc


def prepare_inputs(X, W_ih0, W_hh0, b_ih0, b_hh0, W_ih1, W_hh1, b_ih1, b_hh1,
                   W_br, b_br, n_steps=T):
    """Host-side per-core weight permutation (dies are redundant: core r gets
    the same map as core r%4). Returns in_maps."""
    X = np.asarray(X, np.float32)
    bf = ml_dtypes.bfloat16
    # X^T per step: [T, 128, 4*128] where partition p of chunk k = X row
    # 128k+p; shared by all cores.
    XT = np.ascontiguousarray(
        X[:, :n_steps].transpose(1, 2, 0)        # [T, D, B]
         .reshape(n_steps, 4, P, B)
         .transpose(0, 2, 1, 3)                  # [T, p, k, b]
         .reshape(n_steps, P, 512)).astype(bf)
    maps4 = []
    for s in range(4):
        # local gate rows: [i_lo f_lo g_lo o_lo i_hi f_hi g_hi o_hi] x 128
        cols = np.concatenate(
            [g * H + np.arange(HL * s + P * h, HL * s + P * h + P)
             for h in range(2) for g in range(4)])
        # gather row order: slot k holds core (s^k)'s 256-slice
        perm = np.concatenate(
            [np.arange(HL * (s ^ k), HL * (s ^ k) + HL) for k in range(4)])

        def w(a):
            return np.ascontiguousarray(np.asarray(a, np.float32)).astype(bf)

        b0 = np.asarray(b_ih0 + b_hh0, np.float32)[cols]
        b1 = np.asarray(b_ih1 + b_hh1, np.float32)[cols]
        maps4.append({
            "XT": XT,
            "Wx0": w(np.asarray(W_ih0).T[:, cols]),
            "Wh0": w(np.asarray(W_hh0).T[perm][:, cols]),
            "Wx1": w(np.asarray(W_ih1).T[perm][:, cols]),
            "Wh1": w(np.asarray(W_hh1).T[perm][:, cols]),
            "Wbr": w(np.asarray(W_br).T[perm]),
            "b0p": np.ascontiguousarray(b0.reshape(8, P).T),
            "b1p": np.ascontiguousarray(b1.reshape(8, P).T),
            "bbrp": np.ascontiguousarray(
                np.asarray(b_br, np.float32).reshape(2, P).T),
        })
    return [maps4[r % 4] for r in range(NUM_CORES)]


def collect(results):
    """Full [B, BR] output from the per-core results (core 0 has it all)."""
    return np.ascontiguousarray(
        results[0]["y"].reshape(BR, B).T).astype(np.float32)


_cached_nc = None


def kernel(**inputs):
    global _cached_nc
    if _cached_nc is None:
        _cached_nc = build_program(T)
    in_maps = prepare_inputs(**inputs, n_steps=T)
    res = run_bass_kernel_spmd(_cached_nc, in_maps, list(range(NUM_CORES)))
    return collect(res.results)


# revision 7
# speedup vs baseline: 1.0048x; 1.0048x over previous
"""2-layer LSTM (B=128, T=256, D=512, H=1024) + linear head + ELU on 8 trn2 cores.

Strategy (all hardcoded; v2 "feature-major" design):
  - Everything is computed TRANSPOSED (feature-major): gates^T [1024, B],
    h^T [H, B], c^T [H_local, B]. With the full batch B=128 as the matmul
    moving dim and weights as the 128x128 stationary operand, the PE runs at
    full utilization (the v1 batch-major layout wasted half the array), and
    h^T is directly the rhs of the next step's matmul - no PE transposes.
  - Sharding: the two dies are fully redundant (both compute the whole
    batch); within a die, 4-way tensor-parallel over the hidden dim. Each
    core owns a 256-wide hidden slice = 1024 local gate rows (transposed),
    ordered [i_lo f_lo g_lo o_lo | i_hi f_hi g_hi o_hi] x 128 so each PSUM
    bank holds one i/f/g/o quartet. Per step a core broadcasts its h^T slice
    [256, 128] (bf16, 64KB) to the 3 die peers + itself (XOR-relative slots,
    weight rows pre-permuted on host to match). Zero cross-die traffic.
  - No DRAM round-trips: X^T streams in per-step ([128, 512] bf16 tiles,
    prefetched), xg0/xg1 are fused into the per-step matmul accumulation
    (x-chunks first - for layer 0 they don't wait on the h gather - then
    h-chunks). Layer 1 lags layer 0 by one step so its h0 input is already
    gathered when it runs.
  - Gate biases ride the ACT sigmoid/tanh ops as per-partition bias vectors.
  - Gather buffers are mod-3 rings so a peer's step-t+1 write can never race
    my step-t reads. All weights stay SBUF-resident (bf16, ~60KB/partition).
"""

import sys
from contextlib import ExitStack

import ml_dtypes
import numpy as np

for _p in ("/opt/trn_rl_repo", "/root/.axon_site/_ro/trn_rl_repo"):
    if _p not in sys.path:
        sys.path.append(_p)

import concourse.bacc as bacc
import concourse.mybir as mybir
import concourse.tile as tile
from concourse.bass_utils import run_bass_kernel_spmd
from concourse.tile_rust import add_dep_helper

F32 = mybir.dt.float32
BF16 = mybir.dt.bfloat16
AF = mybir.ActivationFunctionType

P = 128
T = 256
D = 512
H = 1024
BR = 256
B = 128    # full batch on every core (dies are redundant)
HL = 256   # hidden units per core
NL = 1024  # local gate rows per core (i|f|g|o quartets, see cols order)
NUM_CORES = 8


def _build(nc, n_steps):
    """Emit the whole program. Returns (inst, sem, val) waits to patch after
    TileContext exit (remote-arrival waits the scheduler can't model)."""
    xt_in = nc.dram_tensor("XT", [n_steps, P, 512], BF16, kind="ExternalInput").ap()
    wx0_in = nc.dram_tensor("Wx0", [D, NL], BF16, kind="ExternalInput").ap()
    wh0_in = nc.dram_tensor("Wh0", [H, NL], BF16, kind="ExternalInput").ap()
    wx1_in = nc.dram_tensor("Wx1", [H, NL], BF16, kind="ExternalInput").ap()
    wh1_in = nc.dram_tensor("Wh1", [H, NL], BF16, kind="ExternalInput").ap()
    wbr_in = nc.dram_tensor("Wbr", [H, BR], BF16, kind="ExternalInput").ap()
    b0_in = nc.dram_tensor("b0p", [P, 8], F32, kind="ExternalInput").ap()
    b1_in = nc.dram_tensor("b1p", [P, 8], F32, kind="ExternalInput").ap()
    bbr_in = nc.dram_tensor("bbrp", [P, 2], F32, kind="ExternalInput").ap()
    y_out = nc.dram_tensor("y", [2, P, B], F32, kind="ExternalOutput").ap()

    # SBUF-resident weights (feature-major: partition = input-row-in-chunk,
    # free = [k-chunk, local gate col])
    sWx0 = nc.alloc_sbuf_tensor("sWx0", [P, 4, NL], BF16).ap()
    sWh0 = nc.alloc_sbuf_tensor("sWh0", [P, 8, NL], BF16).ap()
    sWx1 = nc.alloc_sbuf_tensor("sWx1", [P, 8, NL], BF16).ap()
    sWh1 = nc.alloc_sbuf_tensor("sWh1", [P, 8, NL], BF16).ap()
    sWbr = nc.alloc_sbuf_tensor("sWbr", [P, 8, BR], BF16).ap()
    sB0 = nc.alloc_sbuf_tensor("sB0", [P, 8], F32).ap()
    sB1 = nc.alloc_sbuf_tensor("sB1", [P, 8], F32).ap()
    sBbr = nc.alloc_sbuf_tensor("sBbr", [P, 2], F32).ap()

    # comm buffers (stable addresses for remote writes); mod-3 rings
    gath = [nc.alloc_sbuf_tensor(f"g{l}", [P, 3, 8, B], BF16).ap() for l in range(2)]
    snd = [nc.alloc_sbuf_tensor(f"s{l}", [P, 3, 2, B], BF16).ap() for l in range(2)]
    cst = [nc.alloc_sbuf_tensor(f"c{l}", [P, 2, B], F32).ap() for l in range(2)]

    rsem = [nc.alloc_semaphore("rsem0"), nc.alloc_semaphore("rsem1")]
    lsem = nc.alloc_semaphore("lsem")

    patches = []

    with tile.TileContext(nc) as tc:
        barrier_nop = nc.gpsimd.nop(nofuse=True)

        # weight loads
        for sb, src, nk in ((sWx0, wx0_in, 4), (sWh0, wh0_in, 8),
                            (sWx1, wx1_in, 8), (sWh1, wh1_in, 8)):
            v = src.rearrange("(k p) n -> k p n", p=P)
            for k in range(nk):
                nc.sync.dma_start(out=sb[:, k], in_=v[k])
        wbrv = wbr_in.rearrange("(k p) n -> k p n", p=P)
        for k in range(8):
            nc.sync.dma_start(out=sWbr[:, k], in_=wbrv[k])
        nc.sync.dma_start(out=sB0, in_=b0_in)
        nc.sync.dma_start(out=sB1, in_=b1_in)
        nc.sync.dma_start(out=sBbr, in_=bbr_in)
        nc.vector.memset(cst[0], 0.0)
        nc.vector.memset(cst[1], 0.0)

        stack = ExitStack()
        ps_pool = stack.enter_context(tc.tile_pool(name="psum", bufs=6, space="PSUM"))
        xt_pool = stack.enter_context(tc.tile_pool(name="xtp", bufs=8))
        tmp_pool = stack.enter_context(tc.tile_pool(name="tmp", bufs=8))
        hd_pool = stack.enter_context(tc.tile_pool(name="hdp", bufs=2))
        first_prep = [None]

        def gated_mms(mms_args, wait):
            """Emit a run of matmuls; the first carries `wait` (a (sem, val)
            runtime patch) and the rest are order-chained to it so the
            scheduler can't hoist any of them above the semaphore wait."""
            first = None
            for out, lhsT, rhs, start, stop in mms_args:
                mm = nc.tensor.matmul(out, lhsT, rhs, start=start, stop=stop)
                if first is None:
                    first = mm
                    if wait is not None:
                        patches.append((mm, wait[0], wait[1]))
                else:
                    add_dep_helper(mm.ins, first.ins, sync=False,
                                   reason="mms chained after gated first")
            return first

        def layer_mms(wx, nkx, xin_tiles, x_wait, wh, g_rhs, h_wait, skip_rec):
            """One layer-step's matmuls: 2 psum quartet tiles [P, 512];
            x-chunks first, then h-chunks. Returns [psA, psB]."""
            pss = [ps_pool.tile([P, 512], F32, name="ps") for _ in range(2)]

            def out_ap(j):
                return pss[j // 4][:, 128 * (j % 4) : 128 * (j % 4 + 1)]

            # Exactly ONE start=True per psum bank tile (j==0 / j==4): a
            # start clears has_written for the whole 2KiB bank row, so a
            # second start in the same bank would wipe sibling regions'
            # accumulation state. Later regions' first matmuls rely on the
            # per-element semantics instead (bit clear -> overwrite + set).
            xargs = []
            for j in range(8):
                for kx in range(nkx):
                    xargs.append((out_ap(j), wx[:, kx, 128 * j : 128 * (j + 1)],
                                  xin_tiles[kx], kx == 0 and j % 4 == 0,
                                  kx == nkx - 1 and skip_rec))
            gated_mms(xargs, x_wait)
            if not skip_rec:
                hargs = []
                for j in range(8):
                    for kh in range(8):
                        hargs.append((out_ap(j), wh[:, kh, 128 * j : 128 * (j + 1)],
                                      g_rhs[:, kh], False, kh == 7))
                gated_mms(hargs, h_wait)
            return pss

        def elem_quartet(ps, half, sB, c, sndt):
            """ps: [P, 512] psum quartet [i f g o] for one 128-half of the
            slice; writes h^T (bf16) into sndt[:, half] and updates c."""
            nc.scalar.activation(ps[:, 0:128], ps[:, 0:128], AF.Sigmoid,
                                 bias=sB[:, 4 * half + 0 : 4 * half + 1])
            nc.scalar.activation(ps[:, 128:256], ps[:, 128:256], AF.Sigmoid,
                                 bias=sB[:, 4 * half + 1 : 4 * half + 2])
            gsb = tmp_pool.tile([P, B], F32, name="gsb")
            nc.scalar.activation(gsb, ps[:, 256:384], AF.Tanh,       # -> SBUF:
                                 bias=sB[:, 4 * half + 2 : 4 * half + 3])
            nc.scalar.activation(ps[:, 384:512], ps[:, 384:512], AF.Sigmoid,
                                 bias=sB[:, 4 * half + 3 : 4 * half + 4])
            t1 = tmp_pool.tile([P, B], F32, name="t1")
            nc.vector.tensor_mul(t1, ps[:, 0:128], gsb)              # i * g
            t2 = tmp_pool.tile([P, B], F32, name="t2")
            nc.vector.tensor_mul(t2, ps[:, 128:256], c[:, half])     # f * c
            nc.vector.tensor_add(c[:, half], t1, t2)
            tcn = tmp_pool.tile([P, B], F32, name="tc")
            nc.scalar.activation(tcn, c[:, half], AF.Tanh)
            nc.vector.tensor_mul(sndt[:, half], ps[:, 384:512], tcn)  # o*tanh(c)

        def send_layer(l, slot, sndt):
            preps = []
            for k in range(4):
                rd = [None] * 8
                rd[k] = (0, k)
                prep = nc.gpsimd.remote_dma_broadcast(
                    gath[l][:, slot, 2 * k : 2 * (k + 1)], sndt,
                    rsem[l], lsem, rdests=rd, queue_num=l)
                preps.append(prep)
                if first_prep[0] is None:
                    first_prep[0] = prep
                    add_dep_helper(prep.ins, barrier_nop.ins, sync=False,
                                   reason="sends after entry barrier")
            trig = nc.gpsimd.trigger_dma(count=None, queue_num=l)
            # the scheduler has no modeled dep between trigger and the desc
            # preps - without these edges it emits the trigger early and the
            # last slot's descriptors only fire on the NEXT step's trigger
            # (costs a full-step stall on every receiver).
            for prep in preps:
                add_dep_helper(trig.ins, prep.ins, sync=False,
                               reason="trigger after all slot preps")

        def layer_step(l, t, s, wx, nkx, xin_tiles, x_wait, sB):
            """l: layer; t: program step (send slot t%3, gathers read slot
            (t-1)%3); s: layer-local time (h_{s-1} is contracted; s==0 skips
            the recurrent phase)."""
            g_rhs = gath[l][:, (t - 1) % 3]
            h_wait = (rsem[l], 8 * s) if s >= 1 else None
            pss = layer_mms(wx, nkx, xin_tiles, x_wait, sWh0 if l == 0 else sWh1,
                            g_rhs, h_wait, s == 0)
            sndt = snd[l][:, t % 3]
            for half in range(2):
                elem_quartet(pss[half], half, sB, cst[l], sndt)
            send_layer(l, t % 3, sndt)

        # ---------------- main loop ----------------
        for t in range(n_steps):
            # layer 0, time t: x-chunks from the streamed X^T tile
            xt = xt_pool.tile([P, 4, B], BF16, name="xt")
            nc.sync.dma_start(out=xt, in_=xt_in[t].rearrange("p (k b) -> p k b", k=4))
            layer_step(0, t, t, sWx0, 4, [xt[:, kx] for kx in range(4)],
                       None, sB0)
            # layer 1, time t-1: x-input = gathered h0_{t-1}
            if t >= 1:
                g0r = gath[0][:, (t - 1) % 3]
                layer_step(1, t, t - 1, sWx1, 8, [g0r[:, k] for k in range(8)],
                           (rsem[0], 8 * t), sB1)

        # tail: layer 1, time n_steps-1 (program step tn = n_steps)
        tn = n_steps
        g0r = gath[0][:, (tn - 1) % 3]
        layer_step(1, tn, tn - 1, sWx1, 8, [g0r[:, k] for k in range(8)],
                   (rsem[0], 8 * tn), sB1)

        # ---------------- head: y^T = ELU(Wbr @ h1_last + bbr) -------------
        g1r = gath[1][:, tn % 3]
        psh = ps_pool.tile([P, 512], F32, name="ps")
        hargs = []
        for jo in range(2):
            for k in range(8):
                hargs.append((psh[:, 128 * jo : 128 * (jo + 1)],
                              sWbr[:, k, 128 * jo : 128 * (jo + 1)],
                              g1r[:, k], k == 0 and jo == 0, k == 7))
        gated_mms(hargs, (rsem[1], 8 * tn))
        for jo in range(2):
            pc = psh[:, 128 * jo : 128 * (jo + 1)]
            xv = hd_pool.tile([P, B], F32, name="xv")
            nc.scalar.activation(xv, pc, AF.Identity, bias=sBbr[:, jo : jo + 1])
            rl = hd_pool.tile([P, B], F32, name="rl")
            nc.vector.tensor_scalar_max(rl, xv, 0.0)
            mn = hd_pool.tile([P, B], F32, name="mn")
            nc.vector.tensor_scalar_min(mn, xv, 0.0)
            ex = hd_pool.tile([P, B], F32, name="ex")
            nc.scalar.activation(ex, mn, AF.Exp)
            s1 = hd_pool.tile([P, B], F32, name="s1")
            nc.vector.tensor_add(s1, rl, ex)
            yv = hd_pool.tile([P, B], F32, name="yv")
            nc.vector.tensor_scalar_add(yv, s1, -1.0)
            nc.sync.dma_start(out=y_out[jo], in_=yv)
        stack.close()

    # post-Tile patches (scheduler can't model remote increments)
    nc._bir_kernel_barrier_sem_replica_groups.append(set(range(NUM_CORES)))
    barrier_nop.wait_op(nc._bir_kernel_barrier_sem, nc.bir_kernel_barrier_sem_inc,
                        "sem-ge", check=False)
    for inst, sem, val in patches:
        if val > 0:
            inst.wait_op(sem, val, "sem-ge", check=False)
    return patches


def build_program(n_steps=T):
    nc = bacc.Bacc("TRN2", target_bir_lowering=False, debug=False,
                   num_devices=NUM_CORES, num_swdge_queues=2)
    _build(nc, n_steps)
    nc.compile()
    return nc


def prepare_inputs(X, W_ih0, W_hh0, b_ih0, b_hh0, W_ih1, W_hh1, b_ih1, b_hh1,
                   W_br, b_br, n_steps=T):
    """Host-side per-core weight permutation (dies are redundant: core r gets
    the same map as core r%4). Returns in_maps."""
    X = np.asarray(X, np.float32)
    bf = ml_dtypes.bfloat16
    # X^T per step: [T, 128, 4*128] where partition p of chunk k = X row
    # 128k+p; shared by all cores.
    XT = np.ascontiguousarray(
        X[:, :n_steps].transpose(1, 2, 0)        # [T, D, B]
         .reshape(n_steps, 4, P, B)
         .transpose(0, 2, 1, 3)                  # [T, p, k, b]
         .reshape(n_steps, P, 512)).astype(bf)
    maps4 = []
    for s in range(4):
        # local gate rows: [i_lo f_lo g_lo o_lo i_hi f_hi g_hi o_hi] x 128
        cols = np.concatenate(
            [g * H + np.arange(HL * s + P * h, HL * s + P * h + P)
             for h in range(2) for g in range(4)])
        # gather row order: slot k holds core (s^k)'s 256-slice
        perm = np.concatenate(
            [np.arange(HL * (s ^ k), HL * (s ^ k) + HL) for k in range(4)])

        def w(a):
            return np.ascontiguousarray(np.asarray(a, np.float32)).astype(bf)

        b0 = np.asarray(b_ih0 + b_hh0, np.float32)[cols]
        b1 = np.asarray(b_ih1 + b_hh1, np.float32)[cols]
        maps4.append({
            "XT": XT,
            "Wx0": w(np.asarray(W_ih0).T[:, cols]),
            "Wh0": w(np.asarray(W_hh0).T[perm][:, cols]),
            "Wx1": w(np.asarray(W_ih1).T[perm][:, cols]),
            "Wh1": w(np.asarray(W_hh1).T[perm][:, cols]),
            "Wbr": w(np.asarray(W_br).T[perm]),
            "b0p": np.ascontiguousarray(b0.reshape(8, P).T),
            "b1p": np.ascontiguousarray(b1.reshape(8, P).T),
            "bbrp": np.ascontiguousarray(
                np.asarray(b_br, np.float32).reshape(2, P).T),
        })
    return [maps4[r % 4] for r in range(NUM_CORES)]


def collect(results):
    """Full [B, BR] output from the per-core results (core 0 has it all)."""
    return np.ascontiguousarray(
        results[0]["y"].reshape(BR, B).T).astype(np.float32)


_cached_nc = None


def kernel(**inputs):
    global _cached_nc
    if _cached_nc is None:
        _cached_nc = build_program(T)
    in_maps = prepare_inputs(**inputs, n_steps=T)
    res = run_bass_kernel_spmd(_cached_nc, in_maps, list(range(NUM_CORES)))
    return collect(res.results)


# revision 17
# speedup vs baseline: 1.1698x; 1.1642x over previous
"""2-layer LSTM (B=128, T=256, D=512, H=1024) + linear head + ELU on 8 trn2 cores.

Strategy (all hardcoded; v6):
  - Feature-major compute: gates^T [1024, B], h^T [H, B], c^T [H_local, B].
    Full batch B=128 as the matmul moving dim, weights as the 128x128
    stationary operand -> full PE utilization, no transposes anywhere.
  - Sharding: dies fully redundant; 4-way tensor-parallel over the hidden dim
    within a die. Core s owns hidden slice [256s, 256s+256) = 1024 local gate
    rows, ordered [i_lo f_lo g_lo o_lo | i_hi f_hi g_hi o_hi] x 128 so each
    PSUM bank holds an i/f/g/o quartet.
  - Communication (the bottleneck - remote-DMA descriptors are processed with
    ~0.5us serial latency per descriptor per lane, so descriptor COUNT rules):
    ONE combined send group per step carrying [h0_t | h1_{t-1}] slices
    together (1KB per partition per dest = half the descriptors of separate
    sends), to the 3 XOR die peers only - the self slice is written in place
    by the elementwise (slot 0 of the gather IS the send source, no loopback
    traffic). Gather buffers are mod-3 rings; layout per slot k:
    [h0_lo h0_hi h1_lo h1_hi] of core (self^k), weight rows pre-permuted on
    host to match. A single semaphore (6 increments per step group) gates
    each step.
  - X^T streams in packed 4 steps per DMA ([128, 2KB] contiguous ->  4x fewer,
    4x bigger descriptors than per-step loads). xg0/xg1 are fused into the
    per-step accumulation (x-chunks first, h-chunks after the gather gate).
    Layer 1 lags layer 0 by one step.
  - Gate biases ride the ACT sigmoid/tanh ops as per-partition bias vectors.
    All weights SBUF-resident (bf16, ~60KB/partition); c state fp32.
"""

import sys
from contextlib import ExitStack

import ml_dtypes
import numpy as np

for _p in ("/opt/trn_rl_repo", "/root/.axon_site/_ro/trn_rl_repo"):
    if _p not in sys.path:
        sys.path.append(_p)

import concourse.bacc as bacc
import concourse.mybir as mybir
import concourse.tile as tile
from concourse.bass_utils import run_bass_kernel_spmd
from concourse.tile_rust import add_dep_helper

F32 = mybir.dt.float32
BF16 = mybir.dt.bfloat16
AF = mybir.ActivationFunctionType

P = 128
T = 256
D = 512
H = 1024
BR = 256
B = 128    # full batch on every core (dies are redundant)
HL = 256   # hidden units per core
NL = 1024  # local gate rows per core
NUM_CORES = 8


def _build(nc, n_steps):
    assert n_steps % 4 == 0
    xt_in = nc.dram_tensor("XT", [n_steps // 4, P, 2048], BF16,
                           kind="ExternalInput").ap()
    wx0_in = nc.dram_tensor("Wx0", [D, NL], BF16, kind="ExternalInput").ap()
    wh0_in = nc.dram_tensor("Wh0", [H, NL], BF16, kind="ExternalInput").ap()
    wx1_in = nc.dram_tensor("Wx1", [H, NL], BF16, kind="ExternalInput").ap()
    wh1_in = nc.dram_tensor("Wh1", [H, NL], BF16, kind="ExternalInput").ap()
    wbr_in = nc.dram_tensor("Wbr", [H, BR], BF16, kind="ExternalInput").ap()
    b0_in = nc.dram_tensor("b0p", [P, 8], F32, kind="ExternalInput").ap()
    b1_in = nc.dram_tensor("b1p", [P, 8], F32, kind="ExternalInput").ap()
    bbr_in = nc.dram_tensor("bbrp", [P, 2], F32, kind="ExternalInput").ap()
    y_out = nc.dram_tensor("y", [2, P, B], F32, kind="ExternalOutput").ap()

    sWx0 = nc.alloc_sbuf_tensor("sWx0", [P, 4, NL], BF16).ap()
    sWh0 = nc.alloc_sbuf_tensor("sWh0", [P, 8, NL], BF16).ap()
    sWx1 = nc.alloc_sbuf_tensor("sWx1", [P, 8, NL], BF16).ap()
    sWh1 = nc.alloc_sbuf_tensor("sWh1", [P, 8, NL], BF16).ap()
    sWbr = nc.alloc_sbuf_tensor("sWbr", [P, 8, BR], BF16).ap()
    sB0 = nc.alloc_sbuf_tensor("sB0", [P, 8], F32).ap()
    sB1 = nc.alloc_sbuf_tensor("sB1", [P, 8], F32).ap()
    sBbr = nc.alloc_sbuf_tensor("sBbr", [P, 2], F32).ap()

    # combined gather ring (stable address for remote writes): slot k holds
    # [h0_lo h0_hi h1_lo h1_hi] of core (self^k); slot 0 (self) doubles as
    # the send source.
    gath = nc.alloc_sbuf_tensor("gath", [P, 3, 16, B], BF16).ap()
    cst = [nc.alloc_sbuf_tensor(f"c{l}", [P, 2, B], F32).ap() for l in range(2)]

    rsem = nc.alloc_semaphore("rsem")
    lsem = nc.alloc_semaphore("lsem")

    patches = []

    def h0c(g, j):   # h0 chunk j view of a gather slot-major tile [P, 16, B]
        return g[:, 4 * (j // 2) + (j % 2)]

    def h1c(g, j):
        return g[:, 4 * (j // 2) + 2 + (j % 2)]

    with tile.TileContext(nc) as tc:
        barrier_nop = nc.gpsimd.nop(nofuse=True)

        for sb, src, nk in ((sWx0, wx0_in, 4), (sWh0, wh0_in, 8),
                            (sWx1, wx1_in, 8), (sWh1, wh1_in, 8)):
            v = src.rearrange("(k p) n -> k p n", p=P)
            for k in range(nk):
                nc.sync.dma_start(out=sb[:, k], in_=v[k])
        wbrv = wbr_in.rearrange("(k p) n -> k p n", p=P)
        for k in range(8):
            nc.sync.dma_start(out=sWbr[:, k], in_=wbrv[k])
        nc.sync.dma_start(out=sB0, in_=b0_in)
        nc.sync.dma_start(out=sB1, in_=b1_in)
        nc.sync.dma_start(out=sBbr, in_=bbr_in)
        nc.vector.memset(cst[0], 0.0)
        nc.vector.memset(cst[1], 0.0)

        stack = ExitStack()
        ps_pool = stack.enter_context(tc.tile_pool(name="psum", bufs=6, space="PSUM"))
        xt_pool = stack.enter_context(tc.tile_pool(name="xtp", bufs=3))
        tmp_pool = stack.enter_context(tc.tile_pool(name="tmp", bufs=8))
        hd_pool = stack.enter_context(tc.tile_pool(name="hdp", bufs=2))
        first_prep = [None]

        def gated_mms(mms_args, wait, chain_to=None):
            """Emit matmuls; the first carries `wait` (runtime patch) unless
            chain_to is given, in which case everything (including the first)
            is order-chained to that instruction instead."""
            first = chain_to
            for out, lhsT, rhs, start, stop in mms_args:
                mm = nc.tensor.matmul(out, lhsT, rhs, start=start, stop=stop)
                if first is None:
                    first = mm
                    if wait is not None:
                        patches.append((mm, wait[0], wait[1]))
                else:
                    add_dep_helper(mm.ins, first.ins, sync=False,
                                   reason="mms chained after gated first")
            return first

        def layer_mms(wx, nkx, xin_tiles, x_wait, wh, h_tiles, h_wait,
                      skip_rec):
            """One layer-step's matmuls into 2 psum quartets. Exactly one
            start=True per psum bank tile (a start clears has_written for the
            whole 2KiB bank row; later regions rely on per-element overwrite
            semantics). If x_wait is set, the first x-matmul carries it and
            everything else chains to it; otherwise h_wait goes on the first
            h-matmul (x-matmuls run ungated - they read DMA-tracked inputs)."""
            pss = [ps_pool.tile([P, 512], F32, name="ps") for _ in range(2)]

            def out_ap(j):
                return pss[j // 4][:, 128 * (j % 4) : 128 * (j % 4 + 1)]

            xargs = []
            for j in range(8):
                for kx in range(nkx):
                    xargs.append((out_ap(j), wx[:, kx, 128 * j : 128 * (j + 1)],
                                  xin_tiles[kx], kx == 0 and j % 4 == 0,
                                  kx == nkx - 1 and skip_rec))
            xgate = gated_mms(xargs, x_wait)
            if not skip_rec:
                hargs = []
                for j in range(8):
                    for kh in range(8):
                        hargs.append((out_ap(j), wh[:, kh, 128 * j : 128 * (j + 1)],
                                      h_tiles[kh], False, kh == 7))
                if x_wait is not None:
                    gated_mms(hargs, None, chain_to=xgate)
                else:
                    gated_mms(hargs, h_wait)
            return pss

        def elem_quartet(ps, half, sB, c, out_bf):
            nc.scalar.activation(ps[:, 0:128], ps[:, 0:128], AF.Sigmoid,
                                 bias=sB[:, 4 * half + 0 : 4 * half + 1])
            nc.scalar.activation(ps[:, 128:256], ps[:, 128:256], AF.Sigmoid,
                                 bias=sB[:, 4 * half + 1 : 4 * half + 2])
            gsb = tmp_pool.tile([P, B], F32, name="gsb")
            nc.scalar.activation(gsb, ps[:, 256:384], AF.Tanh,
                                 bias=sB[:, 4 * half + 2 : 4 * half + 3])
            nc.scalar.activation(ps[:, 384:512], ps[:, 384:512], AF.Sigmoid,
                                 bias=sB[:, 4 * half + 3 : 4 * half + 4])
            t1 = tmp_pool.tile([P, B], F32, name="t1")
            nc.vector.tensor_mul(t1, ps[:, 0:128], gsb)              # i * g
            t2 = tmp_pool.tile([P, B], F32, name="t2")
            nc.vector.tensor_mul(t2, ps[:, 128:256], c[:, half])     # f * c
            nc.vector.tensor_add(c[:, half], t1, t2)
            tcn = tmp_pool.tile([P, B], F32, name="tc")
            nc.scalar.activation(tcn, c[:, half], AF.Tanh)
            nc.vector.tensor_mul(out_bf, ps[:, 384:512], tcn)        # o*tanh(c)

        def send_group(slot):
            """One combined send of gath[:, slot, 0:4] (h0|h1 self slices,
            1KB/partition) to the 3 XOR peers' slot-k regions."""
            src = gath[:, slot, 0:4]
            preps = []
            for k in range(1, 4):
                rd = [None] * 8
                rd[k] = (0, k)
                prep = nc.gpsimd.remote_dma_broadcast(
                    gath[:, slot, 4 * k : 4 * (k + 1)], src,
                    rsem, lsem, rdests=rd, queue_num=0)
                preps.append(prep)
                if first_prep[0] is None:
                    first_prep[0] = prep
                    add_dep_helper(prep.ins, barrier_nop.ins, sync=False,
                                   reason="sends after entry barrier")
            trig = nc.gpsimd.trigger_dma(count=None, queue_num=0)
            for prep in preps:
                add_dep_helper(trig.ins, prep.ins, sync=False,
                               reason="trigger after all slot preps")

        # ---------------- main loop ----------------
        xt4 = None
        for t in range(n_steps):
            if t % 4 == 0:
                xt4 = xt_pool.tile([P, 4, 4, B], BF16, name="xt")
                nc.sync.dma_start(
                    out=xt4,
                    in_=xt_in[t // 4].rearrange("p (s k b) -> p s k b", s=4, k=4))
            gslot = gath[:, t % 3]
            gprev = gath[:, (t - 1) % 3]

            # layer 0, time t
            pss0 = layer_mms(
                sWx0, 4, [xt4[:, t % 4, kx] for kx in range(4)], None,
                sWh0, [h0c(gprev, j) for j in range(8)],
                (rsem, 6 * t), t == 0)
            for half in range(2):
                elem_quartet(pss0[half], half, sB0, cst[0], gslot[:, half])

            # layer 1, time t-1
            if t >= 1:
                pss1 = layer_mms(
                    sWx1, 8, [h0c(gprev, j) for j in range(8)], (rsem, 6 * t),
                    sWh1, [h1c(gprev, j) for j in range(8)],
                    None, t == 1)
                for half in range(2):
                    elem_quartet(pss1[half], half, sB1, cst[1],
                                 gslot[:, 2 + half])

            send_group(t % 3)

        # tail: layer 1, time n_steps-1 (program step tn)
        tn = n_steps
        gprev = gath[:, (tn - 1) % 3]
        gslot = gath[:, tn % 3]
        pss1 = layer_mms(
            sWx1, 8, [h0c(gprev, j) for j in range(8)], (rsem, 6 * tn),
            sWh1, [h1c(gprev, j) for j in range(8)], None, False)
        for half in range(2):
            elem_quartet(pss1[half], half, sB1, cst[1], gslot[:, 2 + half])
        send_group(tn % 3)

        # ---------------- head: y^T = ELU(Wbr @ h1_last + bbr) -------------
        gl = gath[:, tn % 3]
        psh = ps_pool.tile([P, 512], F32, name="ps")
        hargs = []
        for jo in range(2):
            for k in range(8):
                hargs.append((psh[:, 128 * jo : 128 * (jo + 1)],
                              sWbr[:, k, 128 * jo : 128 * (jo + 1)],
                              h1c(gl, k), k == 0 and jo == 0, k == 7))
        gated_mms(hargs, (rsem, 6 * (tn + 1)))
        for jo in range(2):
            pc = psh[:, 128 * jo : 128 * (jo + 1)]
            xv = hd_pool.tile([P, B], F32, name="xv")
            nc.scalar.activation(xv, pc, AF.Identity, bias=sBbr[:, jo : jo + 1])
            rl = hd_pool.tile([P, B], F32, name="rl")
            nc.vector.tensor_scalar_max(rl, xv, 0.0)
            mn = hd_pool.tile([P, B], F32, name="mn")
            nc.vector.tensor_scalar_min(mn, xv, 0.0)
            ex = hd_pool.tile([P, B], F32, name="ex")
            nc.scalar.activation(ex, mn, AF.Exp)
            s1 = hd_pool.tile([P, B], F32, name="s1")
            nc.vector.tensor_add(s1, rl, ex)
            yv = hd_pool.tile([P, B], F32, name="yv")
            nc.vector.tensor_scalar_add(yv, s1, -1.0)
            nc.sync.dma_start(out=y_out[jo], in_=yv)
        stack.close()

    nc._bir_kernel_barrier_sem_replica_groups.append(set(range(NUM_CORES)))
    barrier_nop.wait_op(nc._bir_kernel_barrier_sem, nc.bir_kernel_barrier_sem_inc,
                        "sem-ge", check=False)
    for inst, sem, val in patches:
        if val > 0:
            inst.wait_op(sem, val, "sem-ge", check=False)
    return patches


def build_program(n_steps=T):
    nc = bacc.Bacc("TRN2", target_bir_lowering=False, debug=False,
                   num_devices=NUM_CORES, num_swdge_queues=2)
    _build(nc, n_steps)
    nc.compile()
    return nc


def prepare_inputs(X, W_ih0, W_hh0, b_ih0, b_hh0, W_ih1, W_hh1, b_ih1, b_hh1,
                   W_br, b_br, n_steps=T):
    X = np.asarray(X, np.float32)
    bf = ml_dtypes.bfloat16
    # X^T packed 4 steps per row-block: [T/4, p, (step, k, b)]
    XT = (X[:, :n_steps].transpose(1, 2, 0)         # [T, D, B]
          .reshape(n_steps // 4, 4, 4, P, B)        # [T4, s, k, p, b]
          .transpose(0, 3, 1, 2, 4)                 # [T4, p, s, k, b]
          .reshape(n_steps // 4, P, 2048))
    XT = np.ascontiguousarray(XT).astype(bf)
    maps4 = []
    for s in range(4):
        cols = np.concatenate(
            [g * H + np.arange(HL * s + P * h, HL * s + P * h + P)
             for h in range(2) for g in range(4)])
        perm = np.concatenate(
            [np.arange(HL * (s ^ k), HL * (s ^ k) + HL) for k in range(4)])

        def w(a):
            return np.ascontiguousarray(np.asarray(a, np.float32)).astype(bf)

        b0 = np.asarray(b_ih0 + b_hh0, np.float32)[cols]
        b1 = np.asarray(b_ih1 + b_hh1, np.float32)[cols]
        maps4.append({
            "XT": XT,
            "Wx0": w(np.asarray(W_ih0).T[:, cols]),
            "Wh0": w(np.asarray(W_hh0).T[perm][:, cols]),
            "Wx1": w(np.asarray(W_ih1).T[perm][:, cols]),
            "Wh1": w(np.asarray(W_hh1).T[perm][:, cols]),
            "Wbr": w(np.asarray(W_br).T[perm]),
            "b0p": np.ascontiguousarray(b0.reshape(8, P).T),
            "b1p": np.ascontiguousarray(b1.reshape(8, P).T),
            "bbrp": np.ascontiguousarray(
                np.asarray(b_br, np.float32).reshape(2, P).T),
        })
    return [maps4[r % 4] for r in range(NUM_CORES)]


def collect(results):
    return np.ascontiguousarray(
        results[0]["y"].reshape(BR, B).T).astype(np.float32)


_cached_nc = None


def kernel(**inputs):
    global _cached_nc
    if _cached_nc is None:
        _cached_nc = build_program(T)
    in_maps = prepare_inputs(**inputs, n_steps=T)
    res = run_bass_kernel_spmd(_cached_nc, in_maps, list(range(NUM_CORES)))
    return collect(res.results)


# revision 22
# speedup vs baseline: 1.1713x; 1.0013x over previous
"""2-layer LSTM (B=128, T=256, D=512, H=1024) + linear head + ELU on 8 trn2 cores.

Strategy (all hardcoded; v6):
  - Feature-major compute: gates^T [1024, B], h^T [H, B], c^T [H_local, B].
    Full batch B=128 as the matmul moving dim, weights as the 128x128
    stationary operand -> full PE utilization, no transposes anywhere.
  - Sharding: dies fully redundant; 4-way tensor-parallel over the hidden dim
    within a die. Core s owns hidden slice [256s, 256s+256) = 1024 local gate
    rows, ordered [i_lo f_lo g_lo o_lo | i_hi f_hi g_hi o_hi] x 128 so each
    PSUM bank holds an i/f/g/o quartet.
  - Communication (the bottleneck - remote-DMA descriptors are processed with
    ~0.5us serial latency per descriptor per lane, so descriptor COUNT rules):
    ONE combined send group per step carrying [h0_t | h1_{t-1}] slices
    together (1KB per partition per dest = half the descriptors of separate
    sends), to the 3 XOR die peers only - the self slice is written in place
    by the elementwise (slot 0 of the gather IS the send source, no loopback
    traffic). Gather buffers are mod-3 rings; layout per slot k:
    [h0_lo h0_hi h1_lo h1_hi] of core (self^k), weight rows pre-permuted on
    host to match. A single semaphore (6 increments per step group) gates
    each step.
  - X^T streams in packed 4 steps per DMA ([128, 2KB] contiguous ->  4x fewer,
    4x bigger descriptors than per-step loads). xg0/xg1 are fused into the
    per-step accumulation (x-chunks first, h-chunks after the gather gate).
    Layer 1 lags layer 0 by one step.
  - Gate biases ride the ACT sigmoid/tanh ops as per-partition bias vectors.
    All weights SBUF-resident (bf16, ~60KB/partition); c state fp32.
"""

import sys
from contextlib import ExitStack

import ml_dtypes
import numpy as np

for _p in ("/opt/trn_rl_repo", "/root/.axon_site/_ro/trn_rl_repo"):
    if _p not in sys.path:
        sys.path.append(_p)

import concourse.bacc as bacc
import concourse.mybir as mybir
import concourse.tile as tile
from concourse.bass_utils import run_bass_kernel_spmd
from concourse.tile_rust import add_dep_helper

F32 = mybir.dt.float32
BF16 = mybir.dt.bfloat16
AF = mybir.ActivationFunctionType

P = 128
T = 256
D = 512
H = 1024
BR = 256
B = 128    # full batch on every core (dies are redundant)
HL = 256   # hidden units per core
NL = 1024  # local gate rows per core
NUM_CORES = 8


def _build(nc, n_steps):
    assert n_steps % 4 == 0
    xt_in = nc.dram_tensor("XT", [n_steps // 4, P, 2048], BF16,
                           kind="ExternalInput").ap()
    wx0_in = nc.dram_tensor("Wx0", [D, NL], BF16, kind="ExternalInput").ap()
    wh0_in = nc.dram_tensor("Wh0", [H, NL], BF16, kind="ExternalInput").ap()
    wx1_in = nc.dram_tensor("Wx1", [H, NL], BF16, kind="ExternalInput").ap()
    wh1_in = nc.dram_tensor("Wh1", [H, NL], BF16, kind="ExternalInput").ap()
    wbr_in = nc.dram_tensor("Wbr", [H, BR], BF16, kind="ExternalInput").ap()
    b0_in = nc.dram_tensor("b0p", [P, 8], F32, kind="ExternalInput").ap()
    b1_in = nc.dram_tensor("b1p", [P, 8], F32, kind="ExternalInput").ap()
    bbr_in = nc.dram_tensor("bbrp", [P, 2], F32, kind="ExternalInput").ap()
    y_out = nc.dram_tensor("y", [2, P, B], F32, kind="ExternalOutput").ap()

    sWx0 = nc.alloc_sbuf_tensor("sWx0", [P, 4, NL], BF16).ap()
    sWh0 = nc.alloc_sbuf_tensor("sWh0", [P, 8, NL], BF16).ap()
    sWx1 = nc.alloc_sbuf_tensor("sWx1", [P, 8, NL], BF16).ap()
    sWh1 = nc.alloc_sbuf_tensor("sWh1", [P, 8, NL], BF16).ap()
    sWbr = nc.alloc_sbuf_tensor("sWbr", [P, 8, BR], BF16).ap()
    sB0 = nc.alloc_sbuf_tensor("sB0", [P, 8], F32).ap()
    sB1 = nc.alloc_sbuf_tensor("sB1", [P, 8], F32).ap()
    sBbr = nc.alloc_sbuf_tensor("sBbr", [P, 2], F32).ap()

    # combined gather ring (stable address for remote writes): slot k holds
    # [h0_lo h0_hi h1_lo h1_hi] of core (self^k); slot 0 (self) doubles as
    # the send source.
    gath = nc.alloc_sbuf_tensor("gath", [P, 3, 16, B], BF16).ap()
    cst = [nc.alloc_sbuf_tensor(f"c{l}", [P, 2, B], F32).ap() for l in range(2)]

    rsem = nc.alloc_semaphore("rsem")
    lsem = nc.alloc_semaphore("lsem")

    patches = []

    def h0c(g, j):   # h0 chunk j view of a gather slot-major tile [P, 16, B]
        return g[:, 4 * (j // 2) + (j % 2)]

    def h1c(g, j):
        return g[:, 4 * (j // 2) + 2 + (j % 2)]

    with tile.TileContext(nc) as tc:
        barrier_nop = nc.gpsimd.nop(nofuse=True)

        for sb, src, nk in ((sWx0, wx0_in, 4), (sWh0, wh0_in, 8),
                            (sWx1, wx1_in, 8), (sWh1, wh1_in, 8)):
            v = src.rearrange("(k p) n -> k p n", p=P)
            for k in range(nk):
                nc.sync.dma_start(out=sb[:, k], in_=v[k])
        wbrv = wbr_in.rearrange("(k p) n -> k p n", p=P)
        for k in range(8):
            nc.sync.dma_start(out=sWbr[:, k], in_=wbrv[k])
        nc.sync.dma_start(out=sB0, in_=b0_in)
        nc.sync.dma_start(out=sB1, in_=b1_in)
        nc.sync.dma_start(out=sBbr, in_=bbr_in)
        nc.vector.memset(cst[0], 0.0)
        nc.vector.memset(cst[1], 0.0)

        stack = ExitStack()
        ps_pool = stack.enter_context(tc.tile_pool(name="psum", bufs=6, space="PSUM"))
        xt_pool = stack.enter_context(tc.tile_pool(name="xtp", bufs=3))
        tmp_pool = stack.enter_context(tc.tile_pool(name="tmp", bufs=8))
        hd_pool = stack.enter_context(tc.tile_pool(name="hdp", bufs=2))
        first_prep = [None]

        def gated_mms(mms_args, wait, chain_to=None):
            """Emit matmuls; the first carries `wait` (runtime patch) unless
            chain_to is given, in which case everything (including the first)
            is order-chained to that instruction instead."""
            first = chain_to
            for out, lhsT, rhs, start, stop in mms_args:
                mm = nc.tensor.matmul(out, lhsT, rhs, start=start, stop=stop)
                if first is None:
                    first = mm
                    if wait is not None:
                        patches.append((mm, wait[0], wait[1]))
                else:
                    add_dep_helper(mm.ins, first.ins, sync=False,
                                   reason="mms chained after gated first")
            return first

        def layer_mms(wx, nkx, xin_tiles, x_wait, wh, h_tiles, h_wait,
                      skip_rec):
            """One layer-step's matmuls into 2 psum quartets. Exactly one
            start=True per psum bank tile (a start clears has_written for the
            whole 2KiB bank row; later regions rely on per-element overwrite
            semantics). If x_wait is set, the first x-matmul carries it and
            everything else chains to it; otherwise h_wait goes on the first
            h-matmul (x-matmuls run ungated - they read DMA-tracked inputs)."""
            pss = [ps_pool.tile([P, 512], F32, name="ps") for _ in range(2)]

            def out_ap(j):
                return pss[j // 4][:, 128 * (j % 4) : 128 * (j % 4 + 1)]

            xargs = []
            for j in range(8):
                for kx in range(nkx):
                    xargs.append((out_ap(j), wx[:, kx, 128 * j : 128 * (j + 1)],
                                  xin_tiles[kx], kx == 0 and j % 4 == 0,
                                  kx == nkx - 1 and skip_rec))
            xgate = gated_mms(xargs, x_wait)
            if not skip_rec:
                hargs = []
                for j in range(8):
                    for kh in range(8):
                        hargs.append((out_ap(j), wh[:, kh, 128 * j : 128 * (j + 1)],
                                      h_tiles[kh], False, kh == 7))
                if x_wait is not None:
                    gated_mms(hargs, None, chain_to=xgate)
                else:
                    gated_mms(hargs, h_wait)
            return pss

        def elem_quartet(ps, half, sB, c, out_bf):
            nc.scalar.activation(ps[:, 0:128], ps[:, 0:128], AF.Sigmoid,
                                 bias=sB[:, 4 * half + 0 : 4 * half + 1])
            nc.scalar.activation(ps[:, 128:256], ps[:, 128:256], AF.Sigmoid,
                                 bias=sB[:, 4 * half + 1 : 4 * half + 2])
            gsb = tmp_pool.tile([P, B], F32, name="gsb")
            nc.scalar.activation(gsb, ps[:, 256:384], AF.Tanh,
                                 bias=sB[:, 4 * half + 2 : 4 * half + 3])
            nc.scalar.activation(ps[:, 384:512], ps[:, 384:512], AF.Sigmoid,
                                 bias=sB[:, 4 * half + 3 : 4 * half + 4])
            t1 = tmp_pool.tile([P, B], F32, name="t1")
            nc.vector.tensor_mul(t1, ps[:, 0:128], gsb)              # i * g
            t2 = tmp_pool.tile([P, B], F32, name="t2")
            nc.vector.tensor_mul(t2, ps[:, 128:256], c[:, half])     # f * c
            nc.vector.tensor_add(c[:, half], t1, t2)
            tcn = tmp_pool.tile([P, B], F32, name="tc")
            nc.scalar.activation(tcn, c[:, half], AF.Tanh)
            nc.vector.tensor_mul(out_bf, ps[:, 384:512], tcn)        # o*tanh(c)

        def send_group(slot):
            """One combined send of gath[:, slot, 0:4] (h0|h1 self slices,
            1KB/partition) to the 3 XOR peers' slot-k regions. Calls on one
            SWDGE queue drain serially (~4.3us each), so split them across
            both queues (2+1) to overlap the drains."""
            src = gath[:, slot, 0:4]
            preps = {0: [], 1: []}
            for k in range(1, 4):
                rd = [None] * 8
                rd[k] = (0, k)
                q = 0 if k < 3 else 1
                prep = nc.gpsimd.remote_dma_broadcast(
                    gath[:, slot, 4 * k : 4 * (k + 1)], src,
                    rsem, lsem, rdests=rd, queue_num=q)
                preps[q].append(prep)
                if first_prep[0] is None:
                    first_prep[0] = prep
                    add_dep_helper(prep.ins, barrier_nop.ins, sync=False,
                                   reason="sends after entry barrier")
            for q in (0, 1):
                trig = nc.gpsimd.trigger_dma(count=None, queue_num=q)
                for prep in preps[q]:
                    add_dep_helper(trig.ins, prep.ins, sync=False,
                                   reason="trigger after its queue's preps")

        # ---------------- main loop ----------------
        xt4 = None
        for t in range(n_steps):
            if t % 4 == 0:
                xt4 = xt_pool.tile([P, 4, 4, B], BF16, name="xt")
                nc.sync.dma_start(
                    out=xt4,
                    in_=xt_in[t // 4].rearrange("p (s k b) -> p s k b", s=4, k=4))
            gslot = gath[:, t % 3]
            gprev = gath[:, (t - 1) % 3]

            # layer 0, time t
            pss0 = layer_mms(
                sWx0, 4, [xt4[:, t % 4, kx] for kx in range(4)], None,
                sWh0, [h0c(gprev, j) for j in range(8)],
                (rsem, 6 * t), t == 0)
            for half in range(2):
                elem_quartet(pss0[half], half, sB0, cst[0], gslot[:, half])

            # layer 1, time t-1
            if t >= 1:
                pss1 = layer_mms(
                    sWx1, 8, [h0c(gprev, j) for j in range(8)], (rsem, 6 * t),
                    sWh1, [h1c(gprev, j) for j in range(8)],
                    None, t == 1)
                for half in range(2):
                    elem_quartet(pss1[half], half, sB1, cst[1],
                                 gslot[:, 2 + half])

            send_group(t % 3)

        # tail: layer 1, time n_steps-1 (program step tn)
        tn = n_steps
        gprev = gath[:, (tn - 1) % 3]
        gslot = gath[:, tn % 3]
        pss1 = layer_mms(
            sWx1, 8, [h0c(gprev, j) for j in range(8)], (rsem, 6 * tn),
            sWh1, [h1c(gprev, j) for j in range(8)], None, False)
        for half in range(2):
            elem_quartet(pss1[half], half, sB1, cst[1], gslot[:, 2 + half])
        send_group(tn % 3)

        # ---------------- head: y^T = ELU(Wbr @ h1_last + bbr) -------------
        gl = gath[:, tn % 3]
        psh = ps_pool.tile([P, 512], F32, name="ps")
        hargs = []
        for jo in range(2):
            for k in range(8):
                hargs.append((psh[:, 128 * jo : 128 * (jo + 1)],
                              sWbr[:, k, 128 * jo : 128 * (jo + 1)],
                              h1c(gl, k), k == 0 and jo == 0, k == 7))
        gated_mms(hargs, (rsem, 6 * (tn + 1)))
        for jo in range(2):
            pc = psh[:, 128 * jo : 128 * (jo + 1)]
            xv = hd_pool.tile([P, B], F32, name="xv")
            nc.scalar.activation(xv, pc, AF.Identity, bias=sBbr[:, jo : jo + 1])
            rl = hd_pool.tile([P, B], F32, name="rl")
            nc.vector.tensor_scalar_max(rl, xv, 0.0)
            mn = hd_pool.tile([P, B], F32, name="mn")
            nc.vector.tensor_scalar_min(mn, xv, 0.0)
            ex = hd_pool.tile([P, B], F32, name="ex")
            nc.scalar.activation(ex, mn, AF.Exp)
            s1 = hd_pool.tile([P, B], F32, name="s1")
            nc.vector.tensor_add(s1, rl, ex)
            yv = hd_pool.tile([P, B], F32, name="yv")
            nc.vector.tensor_scalar_add(yv, s1, -1.0)
            nc.sync.dma_start(out=y_out[jo], in_=yv)
        stack.close()

    nc._bir_kernel_barrier_sem_replica_groups.append(set(range(NUM_CORES)))
    barrier_nop.wait_op(nc._bir_kernel_barrier_sem, nc.bir_kernel_barrier_sem_inc,
                        "sem-ge", check=False)
    for inst, sem, val in patches:
        if val > 0:
            inst.wait_op(sem, val, "sem-ge", check=False)
    return patches


def build_program(n_steps=T):
    nc = bacc.Bacc("TRN2", target_bir_lowering=False, debug=False,
                   num_devices=NUM_CORES, num_swdge_queues=2)
    _build(nc, n_steps)
    nc.compile()
    return nc


def prepare_inputs(X, W_ih0, W_hh0, b_ih0, b_hh0, W_ih1, W_hh1, b_ih1, b_hh1,
                   W_br, b_br, n_steps=T):
    X = np.asarray(X, np.float32)
    bf = ml_dtypes.bfloat16
    # X^T packed 4 steps per row-block: [T/4, p, (step, k, b)]
    XT = (X[:, :n_steps].transpose(1, 2, 0)         # [T, D, B]
          .reshape(n_steps // 4, 4, 4, P, B)        # [T4, s, k, p, b]
          .transpose(0, 3, 1, 2, 4)                 # [T4, p, s, k, b]
          .reshape(n_steps // 4, P, 2048))
    XT = np.ascontiguousarray(XT).astype(bf)
    maps4 = []
    for s in range(4):
        cols = np.concatenate(
            [g * H + np.arange(HL * s + P * h, HL * s + P * h + P)
             for h in range(2) for g in range(4)])
        perm = np.concatenate(
            [np.arange(HL * (s ^ k), HL * (s ^ k) + HL) for k in range(4)])

        def w(a):
            return np.ascontiguousarray(np.asarray(a, np.float32)).astype(bf)

        b0 = np.asarray(b_ih0 + b_hh0, np.float32)[cols]
        b1 = np.asarray(b_ih1 + b_hh1, np.float32)[cols]
        maps4.append({
            "XT": XT,
            "Wx0": w(np.asarray(W_ih0).T[:, cols]),
            "Wh0": w(np.asarray(W_hh0).T[perm][:, cols]),
            "Wx1": w(np.asarray(W_ih1).T[perm][:, cols]),
            "Wh1": w(np.asarray(W_hh1).T[perm][:, cols]),
            "Wbr": w(np.asarray(W_br).T[perm]),
            "b0p": np.ascontiguousarray(b0.reshape(8, P).T),
            "b1p": np.ascontiguousarray(b1.reshape(8, P).T),
            "bbrp": np.ascontiguousarray(
                np.asarray(b_br, np.float32).reshape(2, P).T),
        })
    return [maps4[r % 4] for r in range(NUM_CORES)]


def collect(results):
    return np.ascontiguousarray(
        results[0]["y"].reshape(BR, B).T).astype(np.float32)


_cached_nc = None


def kernel(**inputs):
    global _cached_nc
    if _cached_nc is None:
        _cached_nc = build_program(T)
    in_maps = prepare_inputs(**inputs, n_steps=T)
    res = run_bass_kernel_spmd(_cached_nc, in_maps, list(range(NUM_CORES)))
    return collect(res.results)


# revision 23
# speedup vs baseline: 1.2007x; 1.0250x over previous
"""2-layer LSTM (B=128, T=256, D=512, H=1024) + linear head + ELU on 8 trn2 cores.

Strategy (all hardcoded; v6):
  - Feature-major compute: gates^T [1024, B], h^T [H, B], c^T [H_local, B].
    Full batch B=128 as the matmul moving dim, weights as the 128x128
    stationary operand -> full PE utilization, no transposes anywhere.
  - Sharding: dies fully redundant; 4-way tensor-parallel over the hidden dim
    within a die. Core s owns hidden slice [256s, 256s+256) = 1024 local gate
    rows, ordered [i_lo f_lo g_lo o_lo | i_hi f_hi g_hi o_hi] x 128 so each
    PSUM bank holds an i/f/g/o quartet.
  - Communication (the bottleneck - remote-DMA descriptors are processed with
    ~0.5us serial latency per descriptor per lane, so descriptor COUNT rules):
    ONE combined send group per step carrying [h0_t | h1_{t-1}] slices
    together (1KB per partition per dest = half the descriptors of separate
    sends), to the 3 XOR die peers only - the self slice is written in place
    by the elementwise (slot 0 of the gather IS the send source, no loopback
    traffic). Gather buffers are mod-3 rings; layout per slot k:
    [h0_lo h0_hi h1_lo h1_hi] of core (self^k), weight rows pre-permuted on
    host to match. A single semaphore (6 increments per step group) gates
    each step.
  - X^T streams in packed 4 steps per DMA ([128, 2KB] contiguous ->  4x fewer,
    4x bigger descriptors than per-step loads). xg0/xg1 are fused into the
    per-step accumulation (x-chunks first, h-chunks after the gather gate).
    Layer 1 lags layer 0 by one step.
  - Gate biases ride the ACT sigmoid/tanh ops as per-partition bias vectors.
    All weights SBUF-resident (bf16, ~60KB/partition); c state fp32.
"""

import sys
from contextlib import ExitStack

import ml_dtypes
import numpy as np

for _p in ("/opt/trn_rl_repo", "/root/.axon_site/_ro/trn_rl_repo"):
    if _p not in sys.path:
        sys.path.append(_p)

import concourse.bacc as bacc
import concourse.mybir as mybir
import concourse.tile as tile
from concourse.bass_utils import run_bass_kernel_spmd
from concourse.tile_rust import add_dep_helper

F32 = mybir.dt.float32
BF16 = mybir.dt.bfloat16
AF = mybir.ActivationFunctionType

P = 128
T = 256
D = 512
H = 1024
BR = 256
B = 128    # full batch on every core (dies are redundant)
HL = 256   # hidden units per core
NL = 1024  # local gate rows per core
NUM_CORES = 8


def _build(nc, n_steps):
    assert n_steps % 4 == 0
    xt_in = nc.dram_tensor("XT", [n_steps // 4, P, 2048], BF16,
                           kind="ExternalInput").ap()
    wx0_in = nc.dram_tensor("Wx0", [D, NL], BF16, kind="ExternalInput").ap()
    wh0_in = nc.dram_tensor("Wh0", [H, NL], BF16, kind="ExternalInput").ap()
    wx1_in = nc.dram_tensor("Wx1", [H, NL], BF16, kind="ExternalInput").ap()
    wh1_in = nc.dram_tensor("Wh1", [H, NL], BF16, kind="ExternalInput").ap()
    wbr_in = nc.dram_tensor("Wbr", [H, BR], BF16, kind="ExternalInput").ap()
    b0_in = nc.dram_tensor("b0p", [P, 8], F32, kind="ExternalInput").ap()
    b1_in = nc.dram_tensor("b1p", [P, 8], F32, kind="ExternalInput").ap()
    bbr_in = nc.dram_tensor("bbrp", [P, 2], F32, kind="ExternalInput").ap()
    y_out = nc.dram_tensor("y", [2, P, B], F32, kind="ExternalOutput").ap()

    sWx0 = nc.alloc_sbuf_tensor("sWx0", [P, 4, NL], BF16).ap()
    sWh0 = nc.alloc_sbuf_tensor("sWh0", [P, 8, NL], BF16).ap()
    sWx1 = nc.alloc_sbuf_tensor("sWx1", [P, 8, NL], BF16).ap()
    sWh1 = nc.alloc_sbuf_tensor("sWh1", [P, 8, NL], BF16).ap()
    sWbr = nc.alloc_sbuf_tensor("sWbr", [P, 8, BR], BF16).ap()
    sB0 = nc.alloc_sbuf_tensor("sB0", [P, 8], F32).ap()
    sB1 = nc.alloc_sbuf_tensor("sB1", [P, 8], F32).ap()
    sBbr = nc.alloc_sbuf_tensor("sBbr", [P, 2], F32).ap()

    # combined gather ring (stable address for remote writes): slot k holds
    # [h0_lo h0_hi h1_lo h1_hi] of core (self^k); slot 0 (self) doubles as
    # the send source.
    gath = nc.alloc_sbuf_tensor("gath", [P, 3, 16, B], BF16).ap()
    cst = [nc.alloc_sbuf_tensor(f"c{l}", [P, 2, B], F32).ap() for l in range(2)]

    rsem = nc.alloc_semaphore("rsem")
    lsem = nc.alloc_semaphore("lsem")

    patches = []

    def h0c(g, j):   # h0 chunk j view of a gather slot-major tile [P, 16, B]
        return g[:, 4 * (j // 2) + (j % 2)]

    def h1c(g, j):
        return g[:, 4 * (j // 2) + 2 + (j % 2)]

    with tile.TileContext(nc) as tc:
        barrier_nop = nc.gpsimd.nop(nofuse=True)

        for sb, src, nk in ((sWx0, wx0_in, 4), (sWh0, wh0_in, 8),
                            (sWx1, wx1_in, 8), (sWh1, wh1_in, 8)):
            v = src.rearrange("(k p) n -> k p n", p=P)
            for k in range(nk):
                nc.sync.dma_start(out=sb[:, k], in_=v[k])
        wbrv = wbr_in.rearrange("(k p) n -> k p n", p=P)
        for k in range(8):
            nc.sync.dma_start(out=sWbr[:, k], in_=wbrv[k])
        nc.sync.dma_start(out=sB0, in_=b0_in)
        nc.sync.dma_start(out=sB1, in_=b1_in)
        nc.sync.dma_start(out=sBbr, in_=bbr_in)
        nc.vector.memset(cst[0], 0.0)
        nc.vector.memset(cst[1], 0.0)

        stack = ExitStack()
        ps_pool = stack.enter_context(tc.tile_pool(name="psum", bufs=6, space="PSUM"))
        xt_pool = stack.enter_context(tc.tile_pool(name="xtp", bufs=3))
        tmp_pool = stack.enter_context(tc.tile_pool(name="tmp", bufs=8))
        hd_pool = stack.enter_context(tc.tile_pool(name="hdp", bufs=2))
        first_prep = [None]

        def gated_mms(mms_args, wait, chain_to=None):
            """Emit matmuls; the first carries `wait` (runtime patch) unless
            chain_to is given, in which case everything (including the first)
            is order-chained to that instruction instead."""
            first = chain_to
            for out, lhsT, rhs, start, stop in mms_args:
                mm = nc.tensor.matmul(out, lhsT, rhs, start=start, stop=stop)
                if first is None:
                    first = mm
                    if wait is not None:
                        patches.append((mm, wait[0], wait[1]))
                else:
                    add_dep_helper(mm.ins, first.ins, sync=False,
                                   reason="mms chained after gated first")
            return first

        def layer_mms(wx, nkx, xin_tiles, x_wait, wh, h_tiles, h_wait,
                      skip_rec):
            """One layer-step's matmuls into 2 psum quartets. Exactly one
            start=True per psum bank tile (a start clears has_written for the
            whole 2KiB bank row; later regions rely on per-element overwrite
            semantics). If x_wait is set, the first x-matmul carries it and
            everything else chains to it; otherwise h_wait goes on the first
            h-matmul (x-matmuls run ungated - they read DMA-tracked inputs)."""
            pss = [ps_pool.tile([P, 512], F32, name="ps") for _ in range(2)]

            def out_ap(j):
                return pss[j // 4][:, 128 * (j % 4) : 128 * (j % 4 + 1)]

            xargs = []
            for j in range(8):
                for kx in range(nkx):
                    xargs.append((out_ap(j), wx[:, kx, 128 * j : 128 * (j + 1)],
                                  xin_tiles[kx], kx == 0 and j % 4 == 0,
                                  kx == nkx - 1 and skip_rec))
            xgate = gated_mms(xargs, x_wait)
            if not skip_rec:
                def hmm(j, kh, stop):
                    return (out_ap(j), wh[:, kh, 128 * j : 128 * (j + 1)],
                            h_tiles[kh], False, stop)

                # slot-0 (kh 0,1) h-chunks are the locally-produced self slice
                # (Tile tracks the RAW on the gather buffer) - emit them
                # ungated so the PE chews on them during the remote wait.
                a_args = [hmm(j, kh, False) for j in range(8) for kh in (0, 1)]
                b_args = [hmm(j, kh, kh == 7)
                          for j in range(8) for kh in range(2, 8)]
                if x_wait is not None:
                    gated_mms(a_args, None, chain_to=xgate)
                    gated_mms(b_args, None, chain_to=xgate)
                else:
                    gated_mms(a_args, None)
                    gated_mms(b_args, h_wait)
            return pss

        def elem_quartet(ps, half, sB, c, out_bf):
            nc.scalar.activation(ps[:, 0:128], ps[:, 0:128], AF.Sigmoid,
                                 bias=sB[:, 4 * half + 0 : 4 * half + 1])
            nc.scalar.activation(ps[:, 128:256], ps[:, 128:256], AF.Sigmoid,
                                 bias=sB[:, 4 * half + 1 : 4 * half + 2])
            gsb = tmp_pool.tile([P, B], F32, name="gsb")
            nc.scalar.activation(gsb, ps[:, 256:384], AF.Tanh,
                                 bias=sB[:, 4 * half + 2 : 4 * half + 3])
            nc.scalar.activation(ps[:, 384:512], ps[:, 384:512], AF.Sigmoid,
                                 bias=sB[:, 4 * half + 3 : 4 * half + 4])
            t1 = tmp_pool.tile([P, B], F32, name="t1")
            nc.vector.tensor_mul(t1, ps[:, 0:128], gsb)              # i * g
            t2 = tmp_pool.tile([P, B], F32, name="t2")
            nc.vector.tensor_mul(t2, ps[:, 128:256], c[:, half])     # f * c
            nc.vector.tensor_add(c[:, half], t1, t2)
            tcn = tmp_pool.tile([P, B], F32, name="tc")
            nc.scalar.activation(tcn, c[:, half], AF.Tanh)
            nc.vector.tensor_mul(out_bf, ps[:, 384:512], tcn)        # o*tanh(c)

        def send_group(slot):
            """One combined send of gath[:, slot, 0:4] (h0|h1 self slices,
            1KB/partition) to the 3 XOR peers' slot-k regions. Calls on one
            SWDGE queue drain serially (~4.3us each), so split them across
            both queues (2+1) to overlap the drains."""
            src = gath[:, slot, 0:4]
            preps = {0: [], 1: []}
            for k in range(1, 4):
                rd = [None] * 8
                rd[k] = (0, k)
                q = 0 if k < 3 else 1
                prep = nc.gpsimd.remote_dma_broadcast(
                    gath[:, slot, 4 * k : 4 * (k + 1)], src,
                    rsem, lsem, rdests=rd, queue_num=q)
                preps[q].append(prep)
                if first_prep[0] is None:
                    first_prep[0] = prep
                    add_dep_helper(prep.ins, barrier_nop.ins, sync=False,
                                   reason="sends after entry barrier")
            for q in (0, 1):
                trig = nc.gpsimd.trigger_dma(count=None, queue_num=q)
                for prep in preps[q]:
                    add_dep_helper(trig.ins, prep.ins, sync=False,
                                   reason="trigger after its queue's preps")

        # ---------------- main loop ----------------
        xt4 = None
        for t in range(n_steps):
            if t % 4 == 0:
                xt4 = xt_pool.tile([P, 4, 4, B], BF16, name="xt")
                nc.sync.dma_start(
                    out=xt4,
                    in_=xt_in[t // 4].rearrange("p (s k b) -> p s k b", s=4, k=4))
            gslot = gath[:, t % 3]
            gprev = gath[:, (t - 1) % 3]

            # layer 0, time t
            pss0 = layer_mms(
                sWx0, 4, [xt4[:, t % 4, kx] for kx in range(4)], None,
                sWh0, [h0c(gprev, j) for j in range(8)],
                (rsem, 6 * t), t == 0)
            for half in range(2):
                elem_quartet(pss0[half], half, sB0, cst[0], gslot[:, half])

            # layer 1, time t-1
            if t >= 1:
                pss1 = layer_mms(
                    sWx1, 8, [h0c(gprev, j) for j in range(8)], (rsem, 6 * t),
                    sWh1, [h1c(gprev, j) for j in range(8)],
                    None, t == 1)
                for half in range(2):
                    elem_quartet(pss1[half], half, sB1, cst[1],
                                 gslot[:, 2 + half])

            send_group(t % 3)

        # tail: layer 1, time n_steps-1 (program step tn)
        tn = n_steps
        gprev = gath[:, (tn - 1) % 3]
        gslot = gath[:, tn % 3]
        pss1 = layer_mms(
            sWx1, 8, [h0c(gprev, j) for j in range(8)], (rsem, 6 * tn),
            sWh1, [h1c(gprev, j) for j in range(8)], None, False)
        for half in range(2):
            elem_quartet(pss1[half], half, sB1, cst[1], gslot[:, 2 + half])
        send_group(tn % 3)

        # ---------------- head: y^T = ELU(Wbr @ h1_last + bbr) -------------
        gl = gath[:, tn % 3]
        psh = ps_pool.tile([P, 512], F32, name="ps")
        hargs = []
        for jo in range(2):
            for k in range(8):
                hargs.append((psh[:, 128 * jo : 128 * (jo + 1)],
                              sWbr[:, k, 128 * jo : 128 * (jo + 1)],
                              h1c(gl, k), k == 0 and jo == 0, k == 7))
        gated_mms(hargs, (rsem, 6 * (tn + 1)))
        for jo in range(2):
            pc = psh[:, 128 * jo : 128 * (jo + 1)]
            xv = hd_pool.tile([P, B], F32, name="xv")
            nc.scalar.activation(xv, pc, AF.Identity, bias=sBbr[:, jo : jo + 1])
            rl = hd_pool.tile([P, B], F32, name="rl")
            nc.vector.tensor_scalar_max(rl, xv, 0.0)
            mn = hd_pool.tile([P, B], F32, name="mn")
            nc.vector.tensor_scalar_min(mn, xv, 0.0)
            ex = hd_pool.tile([P, B], F32, name="ex")
            nc.scalar.activation(ex, mn, AF.Exp)
            s1 = hd_pool.tile([P, B], F32, name="s1")
            nc.vector.tensor_add(s1, rl, ex)
            yv = hd_pool.tile([P, B], F32, name="yv")
            nc.vector.tensor_scalar_add(yv, s1, -1.0)
            nc.sync.dma_start(out=y_out[jo], in_=yv)
        stack.close()

    nc._bir_kernel_barrier_sem_replica_groups.append(set(range(NUM_CORES)))
    barrier_nop.wait_op(nc._bir_kernel_barrier_sem, nc.bir_kernel_barrier_sem_inc,
                        "sem-ge", check=False)
    for inst, sem, val in patches:
        if val > 0:
            inst.wait_op(sem, val, "sem-ge", check=False)
    return patches


def build_program(n_steps=T):
    nc = bacc.Bacc("TRN2", target_bir_lowering=False, debug=False,
                   num_devices=NUM_CORES, num_swdge_queues=2)
    _build(nc, n_steps)
    nc.compile()
    return nc


def prepare_inputs(X, W_ih0, W_hh0, b_ih0, b_hh0, W_ih1, W_hh1, b_ih1, b_hh1,
                   W_br, b_br, n_steps=T):
    X = np.asarray(X, np.float32)
    bf = ml_dtypes.bfloat16
    # X^T packed 4 steps per row-block: [T/4, p, (step, k, b)]
    XT = (X[:, :n_steps].transpose(1, 2, 0)         # [T, D, B]
          .reshape(n_steps // 4, 4, 4, P, B)        # [T4, s, k, p, b]
          .transpose(0, 3, 1, 2, 4)                 # [T4, p, s, k, b]
          .reshape(n_steps // 4, P, 2048))
    XT = np.ascontiguousarray(XT).astype(bf)
    maps4 = []
    for s in range(4):
        cols = np.concatenate(
            [g * H + np.arange(HL * s + P * h, HL * s + P * h + P)
             for h in range(2) for g in range(4)])
        perm = np.concatenate(
            [np.arange(HL * (s ^ k), HL * (s ^ k) + HL) for k in range(4)])

        def w(a):
            return np.ascontiguousarray(np.asarray(a, np.float32)).astype(bf)

        b0 = np.asarray(b_ih0 + b_hh0, np.float32)[cols]
        b1 = np.asarray(b_ih1 + b_hh1, np.float32)[cols]
        maps4.append({
            "XT": XT,
            "Wx0": w(np.asarray(W_ih0).T[:, cols]),
            "Wh0": w(np.asarray(W_hh0).T[perm][:, cols]),
            "Wx1": w(np.asarray(W_ih1).T[perm][:, cols]),
            "Wh1": w(np.asarray(W_hh1).T[perm][:, cols]),
            "Wbr": w(np.asarray(W_br).T[perm]),
            "b0p": np.ascontiguousarray(b0.reshape(8, P).T),
            "b1p": np.ascontiguousarray(b1.reshape(8, P).T),
            "bbrp": np.ascontiguousarray(
                np.asarray(b_br, np.float32).reshape(2, P).T),
        })
    return [maps4[r % 4] for r in range(NUM_CORES)]


def collect(results):
    return np.ascontiguousarray(
        results[0]["y"].reshape(BR, B).T).astype(np.float32)


_cached_nc = None


def kernel(**inputs):
    global _cached_nc
    if _cached_nc is None:
        _cached_nc = build_program(T)
    in_maps = prepare_inputs(**inputs, n_steps=T)
    res = run_bass_kernel_spmd(_cached_nc, in_maps, list(range(NUM_CORES)))
    return collect(res.results)
